# revision 23
# baseline (speedup 1.0000x reference)
"""Trainium2 Bass kernel for nn_ModelSimplest (4D conv -> relu -> linear -> sigmoid).

fp8 DoubleRow, folded-boff + wavefront ramp + a-paired tails + tuned DMA order.

Per (a, oi): 4212 contraction rows r = (boff, k, l), J-shift baked into SBUF
tiles.  16 full 256-row DR matmuls (u<16) + (even a) one 116x2-row DR tail
matmul pairing (a, a+1) -> 215 matmuls per (t, oi), 384 cols each.

x tile per (t, ia): [128, 17, 2, 6, 64] fp8 — u<16 main units
(rows r = u*256 + g*128 + p), u=16 = tail block (p<116: g=0 rows
(boff=12, kl=208+p) of ia, g=1 same rows of ia+1; zero above).
tfm[a]: [128, 17, 2, 112] — u16 (even a) = tail-pair stationary.

Schedule: wavefront ramp over cells a+oi<5, then a-major main loop.  DMA
issue order keeps all ramp stationaries ahead of later x tiles.
"""
import sys
from contextlib import ExitStack

import numpy as np

sys.path.insert(0, "/opt/trn_rl_repo")

from concourse import bacc, bass, bass_isa, mybir, tile  # noqa: E402
from concourse.bass_utils import run_bass_kernel_spmd  # noqa: E402

KK = 13
S_IN = 18
S_OUT = 6
N_CORES = 8
B_TOTAL = 1024
B_CORE = B_TOTAL // N_CORES
B_SUB = 64
N_SUB = B_CORE // B_SUB
NCH = 3
NM = NCH * S_OUT * S_OUT              # 108
NMP = 112
NROW = KK * S_IN * S_IN               # 4212
NU = 16
NUX = 17                              # 16 main units + tail slot
NTAIL = NROW - NU * 256               # 116
WSCALE = 256.0
NSLOT = 10
RAMP_W = 5
N_DUMMY = 21

F32 = mybir.dt.float32
BF16 = mybir.dt.bfloat16
FP8 = mybir.dt.float8e4
DR = mybir.MatmulPerfMode.DoubleRow

_CACHE = {}


def _build_nc():
    nc = bacc.Bacc(None, target_bir_lowering=False)

    xf = nc.dram_tensor("xf", [N_SUB, S_IN, 128, NUX, 2, S_OUT, B_SUB], FP8,
                        kind="ExternalInput")
    tfm = nc.dram_tensor("tfm", [KK, 128, NUX, 2, NMP], FP8,
                         kind="ExternalInput")
    wl = nc.dram_tensor("wl", [NM, S_OUT * S_OUT, 1], BF16, kind="ExternalInput")
    bias4 = nc.dram_tensor("bias4", [NM, 1], F32, kind="ExternalInput")
    blin = nc.dram_tensor("blin", [1, 1], F32, kind="ExternalInput")
    out = nc.dram_tensor("out", [1, B_CORE], F32, kind="ExternalOutput")

    with tile.TileContext(nc) as tc, ExitStack() as ctx:
        cpool = ctx.enter_context(tc.tile_pool(name="consts", bufs=1))
        wl_sb = cpool.tile([NM, S_OUT * S_OUT, 1], BF16)
        bias_sb = cpool.tile([NM, 1], F32)
        blin_sb = cpool.tile([1, 1], F32)
        ones_sb = cpool.tile([NM, 1], BF16)
        consts_loaded = []

        def load_consts():
            if not consts_loaded:
                nc.scalar.dma_start(wl_sb[:], wl[:])
                nc.scalar.dma_start(bias_sb[:], bias4[:])
                nc.scalar.dma_start(blin_sb[:], blin[:])
                consts_loaded.append(True)

        # HAM warm-up: zero-filled dummy DR matmuls keep the PE busy from
        # the end of the framework preamble (~7.6us) while the first x/tw
        # DMAs are in flight, so the clock gate opens (~3.4us of sustained
        # activity) before the real cells start instead of ~20us in.
        dpool = ctx.enter_context(tc.tile_pool(name="dummy", bufs=1))
        dw = dpool.tile([128, NMP], FP8)
        dx = dpool.tile([128, S_OUT * B_SUB], FP8)
        nc.vector.memset(dw[:], 0)
        nc.vector.memset(dx[:], 0)
        nc.vector.memset(ones_sb[:], 1.0)

        xpool = ctx.enter_context(tc.tile_pool(name="xs", bufs=1))
        twpool = ctx.enter_context(tc.tile_pool(name="tws", bufs=1))
        tw_tiles = {}

        def get_tw(a, split=False, eng=None):
            if a not in tw_tiles:
                eng = eng or nc.sync
                twt = twpool.tile([128, NUX, 2, NMP], FP8, tag=f"tfm{a}",
                                  name=f"tfm{a}")
                if split:
                    eng.dma_start(twt[:, 0:1], tfm[a, :, 0:1])
                    eng.dma_start(twt[:, 1:NUX], tfm[a, :, 1:NUX])
                else:
                    eng.dma_start(twt[:], tfm[a])
                tw_tiles[a] = twt
            return tw_tiles[a]

        x_tiles = {}

        def load_x(t, ia, chunks=None, eng=None):
            if (t, ia) in x_tiles:
                return
            eng = eng or nc.sync
            xt = xpool.tile([128, NUX, 2, S_OUT, B_SUB], FP8,
                            tag=f"x{ia % NSLOT}", name=f"x_{t}_{ia}")
            if chunks:
                for u0, u1 in chunks:
                    eng.dma_start(xt[:, u0:u1], xf[t, ia, :, u0:u1])
            else:
                eng.dma_start(xt[:], xf[t, ia])
            x_tiles[(t, ia)] = xt

        pspool = ctx.enter_context(
            tc.tile_pool(name="ps", bufs=1, space=bass.MemorySpace.PSUM))
        hpool = ctx.enter_context(tc.tile_pool(name="hs", bufs=1))
        opool = ctx.enter_context(tc.tile_pool(name="outs", bufs=2))

        # warm-up dummies: write-only scratch PSUM, zero inputs.
        dscr = pspool.tile([NM, S_OUT * B_SUB], F32, tag="dscr", name="dscr")
        for _ in range(N_DUMMY):
            nc.tensor.matmul(dscr[:], dw[:, 0:NM], dx[:],
                             start=True, stop=True)

        pending = []

        def epilogue_oi(te, pse, i, lgp, first=None, last=None,
                        use_pe=True):
            if first is None:
                first = i == 0
            if last is None:
                last = i == S_OUT - 1

            h = hpool.tile([NM, S_OUT, B_SUB], BF16, tag=f"h{i}",
                           name=f"h{i}_{te}")
            # relu+scale in two j-halves so Act and DVE pipeline; then
            # collapse partitions AND j with a single ones-matmul whose
            # stride-0 PSUM out-AP accumulates the 6 j-slices in place
            # (replaces 6 tiny N=64 matmuls + a vector reduce).
            HJ = S_OUT // 2
            for j0 in (0, HJ):
                nc.scalar.activation(
                    h[:, j0:j0 + HJ, :], pse[i][:, j0:j0 + HJ, :],
                    mybir.ActivationFunctionType.Relu,
                    bias=bias_sb[:],
                )
                nc.vector.tensor_tensor(
                    h[:, j0:j0 + HJ, :], h[:, j0:j0 + HJ, :],
                    wl_sb[:, i * S_OUT + j0:i * S_OUT + j0 + HJ, :]
                    .broadcast_to((NM, HJ, B_SUB)),
                    op=mybir.AluOpType.mult,
                )
            if use_pe:
                nc.tensor.matmul(
                    lgp[:].broadcast_to((1, S_OUT, B_SUB)), ones_sb[:],
                    h[:], start=first, stop=last,
                )
            else:
                # keep the pending epilogue off the PE: partition-reduce on
                # gpsimd, then j-reduce + accumulate on vector
                pr = opool.tile([NM, S_OUT, B_SUB], F32, tag=f"pr{i % 2}",
                                name=f"pr{i}_{te}")
                nc.gpsimd.partition_all_reduce(
                    pr[:], h[:], channels=NM,
                    reduce_op=bass_isa.ReduceOp.add)
                if first:
                    nc.vector.tensor_reduce(
                        lgp[:], pr[0:1].transpose([0, 2, 1]),
                        axis=mybir.AxisListType.X, op=mybir.AluOpType.add)
                else:
                    prj = opool.tile([1, B_SUB], F32, tag=f"prj{i % 2}",
                                     name=f"prj{i}_{te}")
                    nc.vector.tensor_reduce(
                        prj[:], pr[0:1].transpose([0, 2, 1]),
                        axis=mybir.AxisListType.X, op=mybir.AluOpType.add)
                    nc.vector.tensor_tensor(
                        lgp[:], lgp[:], prj[:], op=mybir.AluOpType.add)

        def epilogue_fin(te, lgp):
            src_ap = lgp[:, 0, :] if len(lgp[:].shape) == 3 else lgp[:]
            ot = opool.tile([1, B_SUB], F32, tag="ot", name=f"ot_{te}")
            nc.scalar.activation(
                ot[:], src_ap,
                mybir.ActivationFunctionType.Sigmoid,
                bias=blin_sb[:],
            )
            nc.scalar.dma_start(out[:, te * B_SUB:(te + 1) * B_SUB],
                                ot[:], single_packet=True)

        def emit_epilogue():
            te, pse = pending.pop(0)
            lga = opool.tile([1, B_SUB], F32, tag="lga", name=f"lga_{te}")
            for i in range(S_OUT):
                epilogue_oi(te, pse, i, lga, use_pe=False)
            epilogue_fin(te, lga)

        def cell(t, a, oi, ps):
            xt = x_tiles[(t, a + oi)]
            twt = tw_tiles[a]
            for u in range(NU):
                nc.tensor.matmul(
                    ps[oi][:],
                    twt[:, u, :, 0:NM],
                    xt[:, u, :, :, :],
                    start=(a == 0 and u == 0),
                    stop=False,
                    perf_mode=DR,
                )
            if a % 2 == 0:
                nc.tensor.matmul(
                    ps[oi][:],
                    twt[0:NTAIL, NU, :, 0:NM],
                    xt[0:NTAIL, NU, :, :, :],
                    start=False,
                    stop=(a == KK - 1),
                    perf_mode=DR,
                )

        for t in range(N_SUB):
            ps = [
                pspool.tile([NM, S_OUT, B_SUB], F32, tag=f"ps{i}",
                            name=f"ps{i}_{t}")
                for i in range(S_OUT)
            ]
            if t == 0:
                # Ramp: ONE in-order HW queue (sync), issue strictly by
                # first-need time, fine-chunked so the PE starts on partial
                # tiles.  (Splitting across queues was tried and hurt: the
                # early DMA path is priority-limited, and parallel queues
                # steal bandwidth from the critical first tile.)
                get_tw(0, split=True)
                load_x(0, 0, chunks=[(0, 1), (1, 2), (2, 4), (4, 7),
                                     (7, 11), (11, NUX)])
                load_x(0, 1, chunks=[(0, 6), (6, 12), (12, NUX)])
                get_tw(1)
                load_x(0, 2, chunks=[(0, 6), (6, 12), (12, NUX)])
                get_tw(2)
                load_x(0, 3, chunks=[(0, 9), (9, NUX)])
                get_tw(3)
                load_x(0, 4, chunks=[(0, 9), (9, NUX)])
                get_tw(4)
                load_consts()
                load_x(0, RAMP_W, chunks=[(0, 9), (9, NUX)])
                load_x(0, S_OUT, chunks=[(0, 9), (9, NUX)])
                for ia in range(S_OUT + 1, NSLOT):
                    load_x(0, ia)
            else:
                get_tw(0)
                load_x(t, 0)
                for a in range(1, RAMP_W):
                    get_tw(a)
                    load_x(t, a)
                load_consts()
                load_x(t, RAMP_W)
                get_tw(RAMP_W)
                for ia in range(S_OUT, NSLOT):
                    load_x(t, ia)
            # wavefront ramp; filler dummies between diagonals keep the
            # PE busy across DMA waits so the clock gate never re-throttles
            RAMP_FILL = {0: 10, 1: 5, 2: 2} if t == 0 else {}
            for w in range(RAMP_W):
                for a in range(w + 1):
                    cell(t, a, w - a, ps)
                for _ in range(RAMP_FILL.get(w, 0)):
                    nc.tensor.matmul(dscr[:], dw[:, 0:NM], dx[:],
                                     start=True, stop=True)
                if w == 4 and pending:
                    emit_epilogue()
            # main loop
            for a in range(KK):
                get_tw(a)
                if a + 1 < KK:
                    get_tw(a + 1)
                nxt = a - 1 + NSLOT
                if a >= 1:
                    if nxt < S_IN:
                        load_x(t, nxt)
                    elif t + 1 < N_SUB:
                        load_x(t + 1, nxt - S_IN)
                if t == N_SUB - 1 and a == KK - 1:
                    # final iteration: ps[oi] completes right after cell
                    # (12, oi) — interleave epilogues with the next cells.
                    # oi=5 first so only ONE epilogue chain trails the
                    # last cell.
                    lgp = pspool.tile([1, 1, B_SUB], F32, tag="lg",
                                      name=f"lg_{t}")
                    order = [S_OUT - 1] + list(range(S_OUT - 1))
                    for k, oi in enumerate(order):
                        cell(t, a, oi, ps)
                        epilogue_oi(t, ps, oi, lgp, first=(k == 0),
                                    last=(k == S_OUT - 1))
                    epilogue_fin(t, lgp)
                else:
                    for oi in range(S_OUT):
                        if a + oi >= RAMP_W:
                            cell(t, a, oi, ps)
            if t < N_SUB - 1:
                pending.append((t, ps))

        while pending:
            emit_epilogue()

    nc.compile()
    return nc


try:
    import ml_dtypes
    np_bf16 = ml_dtypes.bfloat16
    np_fp8 = ml_dtypes.float8_e4m3
except ImportError:  # pragma: no cover
    raise


def _prep_inputs(x, W4, b4, Wlin, blin):
    B = x.shape[0]
    r_main = np.arange(NU * 256).reshape(NU, 2, 128)
    boff_m = r_main // 324
    kl_m = r_main % 324
    kl_t = 208 + np.arange(NTAIL)

    xt = np.ascontiguousarray(
        x[:, 0].transpose(3, 4, 1, 2, 0)).reshape(324, S_IN, S_IN, B)
    xt8 = xt.astype(np_fp8)

    # main units: [u, g, p, j, ia, B] -> [ia, p, u, g, j, B]
    jj = boff_m[..., None] + np.arange(S_OUT)
    xm_all = xt8[kl_m[..., None], :, jj, :]
    xm_all = np.ascontiguousarray(xm_all.transpose(4, 2, 0, 1, 3, 5))

    # tail block: [ia, p, g, j, B]
    base = xt8[kl_t, :, 12:12 + S_OUT, :].transpose(1, 0, 2, 3)
    xtl_all = np.zeros((S_IN, NTAIL, 2, S_OUT, B), np_fp8)
    xtl_all[:, :, 0] = base
    xtl_all[:S_IN - 1, :, 1] = base[1:]

    # combined xf [ia, p, u(17), g, j, B]
    xf_all = np.zeros((S_IN, 128, NUX, 2, S_OUT, B), np_fp8)
    xf_all[:, :, :NU] = xm_all
    xf_all[:, :NTAIL, NU] = xtl_all

    T_flat = np.zeros((324, KK, KK, NM), np.float32)
    kl = np.arange(324)
    k_in_v = kl // S_IN
    l_in_v = kl % S_IN
    W4t = W4[:, 0].transpose(0, 3, 4, 1, 2)
    for ch in range(NCH):
        for kp in range(S_OUT):
            for lp in range(S_OUT):
                m = ch * 36 + kp * 6 + lp
                dk = k_in_v - kp
                dl = l_in_v - lp
                valid = (dk >= 0) & (dk < KK) & (dl >= 0) & (dl < KK)
                T_flat[valid, :, :, m] = W4t[ch, dk[valid], dl[valid]]
    Tq = (T_flat * WSCALE).astype(np_fp8)

    tfm_np = np.zeros((KK, 128, NUX, 2, NMP), np_fp8)
    tgt = Tq[kl_m, :, boff_m, :]             # [u, g, p, a, m]
    tfm_np[:, :, :NU, :, :NM] = tgt.transpose(3, 2, 0, 1, 4)
    tailT = Tq[kl_t, :, 12, :]               # [p, a, m]
    for a in range(0, KK, 2):
        tfm_np[a, :NTAIL, NU, 0, :NM] = tailT[:, a]
        if a + 1 < KK:
            tfm_np[a, :NTAIL, NU, 1, :NM] = tailT[:, a + 1]

    m_idx = np.arange(NM)
    ch_idx = m_idx // 36
    rem = m_idx % 36
    i_idx = np.arange(S_OUT)
    j_idx = np.arange(S_OUT)
    feat = (ch_idx[:, None, None] * 1296 + i_idx[None, :, None] * 216
            + j_idx[None, None, :] * 36 + rem[:, None, None])
    wl_np = (Wlin[0, feat].reshape(NM, S_OUT * S_OUT, 1)
             / WSCALE).astype(np_bf16)

    bias4_np = np.ascontiguousarray(
        (b4[m_idx // 36] * WSCALE).astype(np.float32).reshape(NM, 1))
    blin_np = np.asarray(blin, np.float32).reshape(1, 1)
    return xf_all, tfm_np, wl_np, bias4_np, blin_np


def kernel(x, W4, b4, Wlin, blin, _profile=False):
    x = np.asarray(x)
    W4 = np.asarray(W4)
    b4 = np.asarray(b4)
    Wlin = np.asarray(Wlin)
    blin = np.asarray(blin)

    xf_all, tfm_np, wl_np, bias4_np, blin_np = _prep_inputs(
        x, W4, b4, Wlin, blin)

    if "nc" not in _CACHE:
        _CACHE["nc"] = _build_nc()
    nc = _CACHE["nc"]

    in_maps = []
    for core in range(N_CORES):
        b0 = core * B_CORE
        xc = xf_all[..., b0:b0 + B_CORE].reshape(
            S_IN, 128, NUX, 2, S_OUT, N_SUB, B_SUB)
        xc = np.ascontiguousarray(xc.transpose(5, 0, 1, 2, 3, 4, 6))
        in_maps.append({
            "xf": xc,
            "tfm": tfm_np,
            "wl": wl_np,
            "bias4": bias4_np,
            "blin": blin_np,
        })

    res = run_bass_kernel_spmd(
        nc, in_maps, core_ids=list(range(N_CORES)), trace=_profile)
    outs = [res.results[i]["out"].reshape(B_CORE) for i in range(N_CORES)]
    full = np.concatenate(outs).reshape(B_TOTAL, 1).astype(np.float32)
    if _profile:
        return full, res
    return full



# revision 24
# speedup vs baseline: 1.0001x; 1.0001x over previous
"""Trainium2 Bass kernel for nn_ModelSimplest (4D conv -> relu -> linear -> sigmoid).

fp8 DoubleRow, folded-boff + wavefront ramp + a-paired tails + tuned DMA order.

Per (a, oi): 4212 contraction rows r = (boff, k, l), J-shift baked into SBUF
tiles.  16 full 256-row DR matmuls (u<16) + (even a) one 116x2-row DR tail
matmul pairing (a, a+1) -> 215 matmuls per (t, oi), 384 cols each.

x tile per (t, ia): [128, 17, 2, 6, 64] fp8 — u<16 main units
(rows r = u*256 + g*128 + p), u=16 = tail block (p<116: g=0 rows
(boff=12, kl=208+p) of ia, g=1 same rows of ia+1; zero above).
tfm[a]: [128, 17, 2, 112] — u16 (even a) = tail-pair stationary.

Schedule: wavefront ramp over cells a+oi<5, then a-major main loop.  DMA
issue order keeps all ramp stationaries ahead of later x tiles.

Perf notes (trace-driven, ~447us vs ~450us before):
- The PE is ~95% busy at the 162 ns/matmul stream floor (384 cols @2.4GHz
  + NX overhead); the algorithm family is at its optimum (partitions<=128
  force the (13/18)^2 kl zero-padding; Winograd/FFT lose on 18-point
  transform batching; column tiling only helps M<32; PSUM 16KB/partition
  blocks N=768 fusion), so the wins are schedule-level:
- Epilogue: relu (scalar, j-halves) -> x wl broadcast-mult (vector) -> ONE
  ones-matmul per (t, oi) whose stride-0 PSUM out-AP accumulates the 6
  j-slices in place (replaces 6 tiny N=64 matmuls each; ~11us less PE).
  The t=0 ("pending") epilogue runs its partition-reduce on gpsimd
  (partition_all_reduce, ~2.8us/call, latency-irrelevant there).
- HAM warm-up: N_DUMMY zero-input matmuls bridge the ~8us framework
  preamble + first-DMA latency so the PE clock gate opens (K=8/8) at
  ~11.5us and real cells start warm; RAMP_FILL dummies between wavefront
  diagonals absorb residual DMA waits so the gate never re-throttles
  (a >=3.4us idle window would halve the clock for ~7us).
- Ramp DMA: single in-order sync-engine HW queue, issue strictly by first
  need time, tiles u-chunked so matmuls start on partial tiles.  (Dual
  queue via the scalar engine was tried and is SLOWER: the early DMA path
  is serialization-limited, and parallel queues steal bandwidth from the
  critical first tile.)
- Tail: final a=12 processes oi=5 first so only one epilogue chain trails
  the last cell; final out-DMA issues from the scalar engine right after
  its sigmoid.  Remaining tail is ~6us: ~1.3us relu/mult chain + ~3.6us
  intrinsic DMA completion latency (priming does not help).
"""
import sys
from contextlib import ExitStack

import numpy as np

sys.path.insert(0, "/opt/trn_rl_repo")

from concourse import bacc, bass, bass_isa, mybir, tile  # noqa: E402
from concourse.bass_utils import run_bass_kernel_spmd  # noqa: E402

KK = 13
S_IN = 18
S_OUT = 6
N_CORES = 8
B_TOTAL = 1024
B_CORE = B_TOTAL // N_CORES
B_SUB = 64
N_SUB = B_CORE // B_SUB
NCH = 3
NM = NCH * S_OUT * S_OUT              # 108
NMP = 112
NROW = KK * S_IN * S_IN               # 4212
NU = 16
NUX = 17                              # 16 main units + tail slot
NTAIL = NROW - NU * 256               # 116
WSCALE = 256.0
NSLOT = 10
RAMP_W = 5
N_DUMMY = 21

F32 = mybir.dt.float32
BF16 = mybir.dt.bfloat16
FP8 = mybir.dt.float8e4
DR = mybir.MatmulPerfMode.DoubleRow

_CACHE = {}


def _build_nc():
    nc = bacc.Bacc(None, target_bir_lowering=False)

    xf = nc.dram_tensor("xf", [N_SUB, S_IN, 128, NUX, 2, S_OUT, B_SUB], FP8,
                        kind="ExternalInput")
    tfm = nc.dram_tensor("tfm", [KK, 128, NUX, 2, NMP], FP8,
                         kind="ExternalInput")
    wl = nc.dram_tensor("wl", [NM, S_OUT * S_OUT, 1], BF16, kind="ExternalInput")
    bias4 = nc.dram_tensor("bias4", [NM, 1], F32, kind="ExternalInput")
    blin = nc.dram_tensor("blin", [1, 1], F32, kind="ExternalInput")
    out = nc.dram_tensor("out", [1, B_CORE], F32, kind="ExternalOutput")

    with tile.TileContext(nc) as tc, ExitStack() as ctx:
        cpool = ctx.enter_context(tc.tile_pool(name="consts", bufs=1))
        wl_sb = cpool.tile([NM, S_OUT * S_OUT, 1], BF16)
        bias_sb = cpool.tile([NM, 1], F32)
        blin_sb = cpool.tile([1, 1], F32)
        ones_sb = cpool.tile([NM, 1], BF16)
        consts_loaded = []

        def load_consts():
            if not consts_loaded:
                nc.scalar.dma_start(wl_sb[:], wl[:])
                nc.scalar.dma_start(bias_sb[:], bias4[:])
                nc.scalar.dma_start(blin_sb[:], blin[:])
                consts_loaded.append(True)

        # HAM warm-up: zero-filled dummy DR matmuls keep the PE busy from
        # the end of the framework preamble (~7.6us) while the first x/tw
        # DMAs are in flight, so the clock gate opens (~3.4us of sustained
        # activity) before the real cells start instead of ~20us in.
        dpool = ctx.enter_context(tc.tile_pool(name="dummy", bufs=1))
        dw = dpool.tile([128, NMP], FP8)
        dx = dpool.tile([128, S_OUT * B_SUB], FP8)
        nc.vector.memset(dw[:], 0)
        nc.vector.memset(dx[:], 0)
        nc.vector.memset(ones_sb[:], 1.0)

        xpool = ctx.enter_context(tc.tile_pool(name="xs", bufs=1))
        twpool = ctx.enter_context(tc.tile_pool(name="tws", bufs=1))
        tw_tiles = {}

        def get_tw(a, split=False, eng=None):
            if a not in tw_tiles:
                eng = eng or nc.sync
                twt = twpool.tile([128, NUX, 2, NMP], FP8, tag=f"tfm{a}",
                                  name=f"tfm{a}")
                if split:
                    eng.dma_start(twt[:, 0:1], tfm[a, :, 0:1])
                    eng.dma_start(twt[:, 1:NUX], tfm[a, :, 1:NUX])
                else:
                    eng.dma_start(twt[:], tfm[a])
                tw_tiles[a] = twt
            return tw_tiles[a]

        x_tiles = {}

        def load_x(t, ia, chunks=None, eng=None):
            if (t, ia) in x_tiles:
                return
            eng = eng or nc.sync
            xt = xpool.tile([128, NUX, 2, S_OUT, B_SUB], FP8,
                            tag=f"x{ia % NSLOT}", name=f"x_{t}_{ia}")
            if chunks:
                for u0, u1 in chunks:
                    eng.dma_start(xt[:, u0:u1], xf[t, ia, :, u0:u1])
            else:
                eng.dma_start(xt[:], xf[t, ia])
            x_tiles[(t, ia)] = xt

        pspool = ctx.enter_context(
            tc.tile_pool(name="ps", bufs=1, space=bass.MemorySpace.PSUM))
        hpool = ctx.enter_context(tc.tile_pool(name="hs", bufs=1))
        opool = ctx.enter_context(tc.tile_pool(name="outs", bufs=2))

        # warm-up dummies: write-only scratch PSUM, zero inputs.
        dscr = pspool.tile([NM, S_OUT * B_SUB], F32, tag="dscr", name="dscr")
        for _ in range(N_DUMMY):
            nc.tensor.matmul(dscr[:], dw[:, 0:NM], dx[:],
                             start=True, stop=True)

        pending = []

        def epilogue_oi(te, pse, i, lgp, first=None, last=None,
                        use_pe=True):
            if first is None:
                first = i == 0
            if last is None:
                last = i == S_OUT - 1

            h = hpool.tile([NM, S_OUT, B_SUB], BF16, tag=f"h{i}",
                           name=f"h{i}_{te}")
            # relu+scale in two j-halves so Act and DVE pipeline; then
            # collapse partitions AND j with a single ones-matmul whose
            # stride-0 PSUM out-AP accumulates the 6 j-slices in place
            # (replaces 6 tiny N=64 matmuls + a vector reduce).
            HJ = S_OUT // 2
            for j0 in (0, HJ):
                nc.scalar.activation(
                    h[:, j0:j0 + HJ, :], pse[i][:, j0:j0 + HJ, :],
                    mybir.ActivationFunctionType.Relu,
                    bias=bias_sb[:],
                )
                nc.vector.tensor_tensor(
                    h[:, j0:j0 + HJ, :], h[:, j0:j0 + HJ, :],
                    wl_sb[:, i * S_OUT + j0:i * S_OUT + j0 + HJ, :]
                    .broadcast_to((NM, HJ, B_SUB)),
                    op=mybir.AluOpType.mult,
                )
            if use_pe:
                nc.tensor.matmul(
                    lgp[:].broadcast_to((1, S_OUT, B_SUB)), ones_sb[:],
                    h[:], start=first, stop=last,
                )
            else:
                # keep the pending epilogue off the PE: partition-reduce on
                # gpsimd, then j-reduce + accumulate on vector
                pr = opool.tile([NM, S_OUT, B_SUB], F32, tag=f"pr{i % 2}",
                                name=f"pr{i}_{te}")
                nc.gpsimd.partition_all_reduce(
                    pr[:], h[:], channels=NM,
                    reduce_op=bass_isa.ReduceOp.add)
                if first:
                    nc.vector.tensor_reduce(
                        lgp[:], pr[0:1].transpose([0, 2, 1]),
                        axis=mybir.AxisListType.X, op=mybir.AluOpType.add)
                else:
                    prj = opool.tile([1, B_SUB], F32, tag=f"prj{i % 2}",
                                     name=f"prj{i}_{te}")
                    nc.vector.tensor_reduce(
                        prj[:], pr[0:1].transpose([0, 2, 1]),
                        axis=mybir.AxisListType.X, op=mybir.AluOpType.add)
                    nc.vector.tensor_tensor(
                        lgp[:], lgp[:], prj[:], op=mybir.AluOpType.add)

        def epilogue_fin(te, lgp):
            src_ap = lgp[:, 0, :] if len(lgp[:].shape) == 3 else lgp[:]
            ot = opool.tile([1, B_SUB], F32, tag="ot", name=f"ot_{te}")
            nc.scalar.activation(
                ot[:], src_ap,
                mybir.ActivationFunctionType.Sigmoid,
                bias=blin_sb[:],
            )
            nc.scalar.dma_start(out[:, te * B_SUB:(te + 1) * B_SUB],
                                ot[:], single_packet=True)

        def emit_epilogue():
            te, pse = pending.pop(0)
            lga = opool.tile([1, B_SUB], F32, tag="lga", name=f"lga_{te}")
            for i in range(S_OUT):
                epilogue_oi(te, pse, i, lga, use_pe=False)
            epilogue_fin(te, lga)

        def cell(t, a, oi, ps):
            xt = x_tiles[(t, a + oi)]
            twt = tw_tiles[a]
            for u in range(NU):
                nc.tensor.matmul(
                    ps[oi][:],
                    twt[:, u, :, 0:NM],
                    xt[:, u, :, :, :],
                    start=(a == 0 and u == 0),
                    stop=False,
                    perf_mode=DR,
                )
            if a % 2 == 0:
                nc.tensor.matmul(
                    ps[oi][:],
                    twt[0:NTAIL, NU, :, 0:NM],
                    xt[0:NTAIL, NU, :, :, :],
                    start=False,
                    stop=(a == KK - 1),
                    perf_mode=DR,
                )

        for t in range(N_SUB):
            ps = [
                pspool.tile([NM, S_OUT, B_SUB], F32, tag=f"ps{i}",
                            name=f"ps{i}_{t}")
                for i in range(S_OUT)
            ]
            if t == 0:
                # Ramp: ONE in-order HW queue (sync), issue strictly by
                # first-need time, fine-chunked so the PE starts on partial
                # tiles.  (Splitting across queues was tried and hurt: the
                # early DMA path is priority-limited, and parallel queues
                # steal bandwidth from the critical first tile.)
                get_tw(0, split=True)
                load_x(0, 0, chunks=[(0, 1), (1, 2), (2, 4), (4, 7),
                                     (7, 11), (11, NUX)])
                load_x(0, 1, chunks=[(0, 6), (6, 12), (12, NUX)])
                get_tw(1)
                load_x(0, 2, chunks=[(0, 6), (6, 12), (12, NUX)])
                get_tw(2)
                load_x(0, 3, chunks=[(0, 9), (9, NUX)])
                get_tw(3)
                load_x(0, 4, chunks=[(0, 9), (9, NUX)])
                get_tw(4)
                load_consts()
                load_x(0, RAMP_W, chunks=[(0, 9), (9, NUX)])
                load_x(0, S_OUT, chunks=[(0, 9), (9, NUX)])
                for ia in range(S_OUT + 1, NSLOT):
                    load_x(0, ia)
            else:
                get_tw(0)
                load_x(t, 0)
                for a in range(1, RAMP_W):
                    get_tw(a)
                    load_x(t, a)
                load_consts()
                load_x(t, RAMP_W)
                get_tw(RAMP_W)
                for ia in range(S_OUT, NSLOT):
                    load_x(t, ia)
            # wavefront ramp; filler dummies between diagonals keep the
            # PE busy across DMA waits so the clock gate never re-throttles
            RAMP_FILL = {0: 10, 1: 5, 2: 2} if t == 0 else {}
            for w in range(RAMP_W):
                for a in range(w + 1):
                    cell(t, a, w - a, ps)
                for _ in range(RAMP_FILL.get(w, 0)):
                    nc.tensor.matmul(dscr[:], dw[:, 0:NM], dx[:],
                                     start=True, stop=True)
                if w == 4 and pending:
                    emit_epilogue()
            # main loop
            for a in range(KK):
                get_tw(a)
                if a + 1 < KK:
                    get_tw(a + 1)
                nxt = a - 1 + NSLOT
                if a >= 1:
                    if nxt < S_IN:
                        load_x(t, nxt)
                    elif t + 1 < N_SUB:
                        load_x(t + 1, nxt - S_IN)
                if t == N_SUB - 1 and a == KK - 1:
                    # final iteration: ps[oi] completes right after cell
                    # (12, oi) — interleave epilogues with the next cells.
                    # oi=5 first so only ONE epilogue chain trails the
                    # last cell.
                    lgp = pspool.tile([1, 1, B_SUB], F32, tag="lg",
                                      name=f"lg_{t}")
                    order = [S_OUT - 1] + list(range(S_OUT - 1))
                    for k, oi in enumerate(order):
                        cell(t, a, oi, ps)
                        epilogue_oi(t, ps, oi, lgp, first=(k == 0),
                                    last=(k == S_OUT - 1))
                    epilogue_fin(t, lgp)
                else:
                    for oi in range(S_OUT):
                        if a + oi >= RAMP_W:
                            cell(t, a, oi, ps)
            if t < N_SUB - 1:
                pending.append((t, ps))

        while pending:
            emit_epilogue()

    nc.compile()
    return nc


try:
    import ml_dtypes
    np_bf16 = ml_dtypes.bfloat16
    np_fp8 = ml_dtypes.float8_e4m3
except ImportError:  # pragma: no cover
    raise


def _prep_inputs(x, W4, b4, Wlin, blin):
    B = x.shape[0]
    r_main = np.arange(NU * 256).reshape(NU, 2, 128)
    boff_m = r_main // 324
    kl_m = r_main % 324
    kl_t = 208 + np.arange(NTAIL)

    xt = np.ascontiguousarray(
        x[:, 0].transpose(3, 4, 1, 2, 0)).reshape(324, S_IN, S_IN, B)
    xt8 = xt.astype(np_fp8)

    # main units: [u, g, p, j, ia, B] -> [ia, p, u, g, j, B]
    jj = boff_m[..., None] + np.arange(S_OUT)
    xm_all = xt8[kl_m[..., None], :, jj, :]
    xm_all = np.ascontiguousarray(xm_all.transpose(4, 2, 0, 1, 3, 5))

    # tail block: [ia, p, g, j, B]
    base = xt8[kl_t, :, 12:12 + S_OUT, :].transpose(1, 0, 2, 3)
    xtl_all = np.zeros((S_IN, NTAIL, 2, S_OUT, B), np_fp8)
    xtl_all[:, :, 0] = base
    xtl_all[:S_IN - 1, :, 1] = base[1:]

    # combined xf [ia, p, u(17), g, j, B]
    xf_all = np.zeros((S_IN, 128, NUX, 2, S_OUT, B), np_fp8)
    xf_all[:, :, :NU] = xm_all
    xf_all[:, :NTAIL, NU] = xtl_all

    T_flat = np.zeros((324, KK, KK, NM), np.float32)
    kl = np.arange(324)
    k_in_v = kl // S_IN
    l_in_v = kl % S_IN
    W4t = W4[:, 0].transpose(0, 3, 4, 1, 2)
    for ch in range(NCH):
        for kp in range(S_OUT):
            for lp in range(S_OUT):
                m = ch * 36 + kp * 6 + lp
                dk = k_in_v - kp
                dl = l_in_v - lp
                valid = (dk >= 0) & (dk < KK) & (dl >= 0) & (dl < KK)
                T_flat[valid, :, :, m] = W4t[ch, dk[valid], dl[valid]]
    Tq = (T_flat * WSCALE).astype(np_fp8)

    tfm_np = np.zeros((KK, 128, NUX, 2, NMP), np_fp8)
    tgt = Tq[kl_m, :, boff_m, :]             # [u, g, p, a, m]
    tfm_np[:, :, :NU, :, :NM] = tgt.transpose(3, 2, 0, 1, 4)
    tailT = Tq[kl_t, :, 12, :]               # [p, a, m]
    for a in range(0, KK, 2):
        tfm_np[a, :NTAIL, NU, 0, :NM] = tailT[:, a]
        if a + 1 < KK:
            tfm_np[a, :NTAIL, NU, 1, :NM] = tailT[:, a + 1]

    m_idx = np.arange(NM)
    ch_idx = m_idx // 36
    rem = m_idx % 36
    i_idx = np.arange(S_OUT)
    j_idx = np.arange(S_OUT)
    feat = (ch_idx[:, None, None] * 1296 + i_idx[None, :, None] * 216
            + j_idx[None, None, :] * 36 + rem[:, None, None])
    wl_np = (Wlin[0, feat].reshape(NM, S_OUT * S_OUT, 1)
             / WSCALE).astype(np_bf16)

    bias4_np = np.ascontiguousarray(
        (b4[m_idx // 36] * WSCALE).astype(np.float32).reshape(NM, 1))
    blin_np = np.asarray(blin, np.float32).reshape(1, 1)
    return xf_all, tfm_np, wl_np, bias4_np, blin_np


def kernel(x, W4, b4, Wlin, blin, _profile=False):
    x = np.asarray(x)
    W4 = np.asarray(W4)
    b4 = np.asarray(b4)
    Wlin = np.asarray(Wlin)
    blin = np.asarray(blin)

    xf_all, tfm_np, wl_np, bias4_np, blin_np = _prep_inputs(
        x, W4, b4, Wlin, blin)

    if "nc" not in _CACHE:
        _CACHE["nc"] = _build_nc()
    nc = _CACHE["nc"]

    in_maps = []
    for core in range(N_CORES):
        b0 = core * B_CORE
        xc = xf_all[..., b0:b0 + B_CORE].reshape(
            S_IN, 128, NUX, 2, S_OUT, N_SUB, B_SUB)
        xc = np.ascontiguousarray(xc.transpose(5, 0, 1, 2, 3, 4, 6))
        in_maps.append({
            "xf": xc,
            "tfm": tfm_np,
            "wl": wl_np,
            "bias4": bias4_np,
            "blin": blin_np,
        })

    res = run_bass_kernel_spmd(
        nc, in_maps, core_ids=list(range(N_CORES)), trace=_profile)
    outs = [res.results[i]["out"].reshape(B_CORE) for i in range(N_CORES)]
    full = np.concatenate(outs).reshape(B_TOTAL, 1).astype(np.float32)
    if _profile:
        return full, res
    return full



# revision 25
# speedup vs baseline: 1.0006x; 1.0005x over previous
"""Trainium2 Bass kernel for nn_ModelSimplest (4D conv -> relu -> linear -> sigmoid).

fp8 DoubleRow, folded-boff + wavefront ramp + a-paired tails + tuned DMA order.

Per (a, oi): 4212 contraction rows r = (boff, k, l), J-shift baked into SBUF
tiles.  16 full 256-row DR matmuls (u<16) + (even a) one 116x2-row DR tail
matmul pairing (a, a+1) -> 215 matmuls per (t, oi), 384 cols each.

x tile per (t, ia): [128, 17, 2, 6, 64] fp8 — u<16 main units
(rows r = u*256 + g*128 + p), u=16 = tail block (p<116: g=0 rows
(boff=12, kl=208+p) of ia, g=1 same rows of ia+1; zero above).
tfm[a]: [128, 17, 2, 112] — u16 (even a) = tail-pair stationary.

Schedule: wavefront ramp over cells a+oi<5, then a-major main loop.  DMA
issue order keeps all ramp stationaries ahead of later x tiles.

Perf notes (trace-driven, ~447us vs ~450us before):
- The PE is ~95% busy at the 162 ns/matmul stream floor (384 cols @2.4GHz
  + NX overhead); the algorithm family is at its optimum (partitions<=128
  force the (13/18)^2 kl zero-padding; Winograd/FFT lose on 18-point
  transform batching; column tiling only helps M<32; PSUM 16KB/partition
  blocks N=768 fusion), so the wins are schedule-level:
- Epilogue: relu (scalar, j-halves) -> x wl broadcast-mult (vector) -> ONE
  ones-matmul per (t, oi) whose stride-0 PSUM out-AP accumulates the 6
  j-slices in place (replaces 6 tiny N=64 matmuls each; ~11us less PE).
  The t=0 ("pending") epilogue runs its partition-reduce on gpsimd
  (partition_all_reduce, ~2.8us/call, latency-irrelevant there).
- HAM warm-up: N_DUMMY zero-input matmuls bridge the ~8us framework
  preamble + first-DMA latency so the PE clock gate opens (K=8/8) at
  ~11.5us and real cells start warm; RAMP_FILL dummies between wavefront
  diagonals absorb residual DMA waits so the gate never re-throttles
  (a >=3.4us idle window would halve the clock for ~7us).
- Ramp DMA: single in-order sync-engine HW queue, issue strictly by first
  need time, tiles u-chunked so matmuls start on partial tiles.  (Dual
  queue via the scalar engine was tried and is SLOWER: the early DMA path
  is serialization-limited, and parallel queues steal bandwidth from the
  critical first tile.)
- Tail: final a=12 processes oi=5 first so only one epilogue chain trails
  the last cell; final out-DMA issues from the scalar engine right after
  its sigmoid.  Remaining tail is ~6us: ~1.3us relu/mult chain + ~3.6us
  intrinsic DMA completion latency (priming does not help).
"""
import sys
from contextlib import ExitStack

import numpy as np

sys.path.insert(0, "/opt/trn_rl_repo")

from concourse import bacc, bass, bass_isa, mybir, tile  # noqa: E402
from concourse.bass_utils import run_bass_kernel_spmd  # noqa: E402

KK = 13
S_IN = 18
S_OUT = 6
N_CORES = 8
B_TOTAL = 1024
B_CORE = B_TOTAL // N_CORES
B_SUB = 64
N_SUB = B_CORE // B_SUB
NCH = 3
NM = NCH * S_OUT * S_OUT              # 108
NMP = 112
NROW = KK * S_IN * S_IN               # 4212
NU = 16
NUX = 17                              # 16 main units + tail slot
NTAIL = NROW - NU * 256               # 116
WSCALE = 256.0
NSLOT = 10
RAMP_W = 5
N_DUMMY = 21

F32 = mybir.dt.float32
BF16 = mybir.dt.bfloat16
FP8 = mybir.dt.float8e4
DR = mybir.MatmulPerfMode.DoubleRow

_CACHE = {}


def _build_nc():
    nc = bacc.Bacc(None, target_bir_lowering=False)

    xf = nc.dram_tensor("xf", [N_SUB, S_IN, 128, NUX, 2, S_OUT, B_SUB], FP8,
                        kind="ExternalInput")
    tfm = nc.dram_tensor("tfm", [KK, 128, NUX, 2, NMP], FP8,
                         kind="ExternalInput")
    wl = nc.dram_tensor("wl", [NM, S_OUT * S_OUT, 1], BF16, kind="ExternalInput")
    bias4 = nc.dram_tensor("bias4", [NM, 1], F32, kind="ExternalInput")
    blin = nc.dram_tensor("blin", [1, 1], F32, kind="ExternalInput")
    out = nc.dram_tensor("out", [1, B_CORE], F32, kind="ExternalOutput")

    with tile.TileContext(nc) as tc, ExitStack() as ctx:
        cpool = ctx.enter_context(tc.tile_pool(name="consts", bufs=1))
        wl_sb = cpool.tile([NM, S_OUT * S_OUT, 1], BF16)
        bias_sb = cpool.tile([NM, 1], F32)
        blin_sb = cpool.tile([1, 1], F32)
        ones_sb = cpool.tile([NM, 1], BF16)
        consts_loaded = []

        def load_consts():
            if not consts_loaded:
                nc.scalar.dma_start(wl_sb[:], wl[:])
                nc.scalar.dma_start(bias_sb[:], bias4[:])
                nc.scalar.dma_start(blin_sb[:], blin[:])
                consts_loaded.append(True)

        # HAM warm-up: zero-filled dummy DR matmuls keep the PE busy from
        # the end of the framework preamble (~7.6us) while the first x/tw
        # DMAs are in flight, so the clock gate opens (~3.4us of sustained
        # activity) before the real cells start instead of ~20us in.
        dpool = ctx.enter_context(tc.tile_pool(name="dummy", bufs=1))
        dw = dpool.tile([128, NMP], FP8)
        dx = dpool.tile([128, S_OUT * B_SUB], FP8)
        nc.vector.memset(dw[:], 0)
        nc.vector.memset(dx[:], 0)
        nc.vector.memset(ones_sb[:], 1.0)

        xpool = ctx.enter_context(tc.tile_pool(name="xs", bufs=1))
        twpool = ctx.enter_context(tc.tile_pool(name="tws", bufs=1))
        tw_tiles = {}

        def get_tw(a, split=False, eng=None):
            if a not in tw_tiles:
                eng = eng or nc.sync
                twt = twpool.tile([128, NUX, 2, NMP], FP8, tag=f"tfm{a}",
                                  name=f"tfm{a}")
                if split:
                    eng.dma_start(twt[:, 0:1], tfm[a, :, 0:1])
                    eng.dma_start(twt[:, 1:NUX], tfm[a, :, 1:NUX])
                else:
                    eng.dma_start(twt[:], tfm[a])
                tw_tiles[a] = twt
            return tw_tiles[a]

        x_tiles = {}

        def load_x(t, ia, chunks=None, eng=None):
            if (t, ia) in x_tiles:
                return
            eng = eng or nc.sync
            xt = xpool.tile([128, NUX, 2, S_OUT, B_SUB], FP8,
                            tag=f"x{ia % NSLOT}", name=f"x_{t}_{ia}")
            if chunks:
                for u0, u1 in chunks:
                    eng.dma_start(xt[:, u0:u1], xf[t, ia, :, u0:u1])
            else:
                eng.dma_start(xt[:], xf[t, ia])
            x_tiles[(t, ia)] = xt

        pspool = ctx.enter_context(
            tc.tile_pool(name="ps", bufs=1, space=bass.MemorySpace.PSUM))
        hpool = ctx.enter_context(tc.tile_pool(name="hs", bufs=1))
        opool = ctx.enter_context(tc.tile_pool(name="outs", bufs=2))

        # warm-up dummies: write-only scratch PSUM, zero inputs.
        dscr = pspool.tile([NM, S_OUT * B_SUB], F32, tag="dscr", name="dscr")
        for _ in range(N_DUMMY):
            nc.tensor.matmul(dscr[:], dw[:, 0:NM], dx[:],
                             start=True, stop=True)

        pending = []

        def epilogue_oi(te, pse, i, lgp, first=None, last=None,
                        use_pe=True, split_mm=False):
            if first is None:
                first = i == 0
            if last is None:
                last = i == S_OUT - 1

            h = hpool.tile([NM, S_OUT, B_SUB], BF16, tag=f"h{i}",
                           name=f"h{i}_{te}")
            # relu+scale in two j-halves so Act and DVE pipeline; then
            # collapse partitions AND j with a single ones-matmul whose
            # stride-0 PSUM out-AP accumulates the 6 j-slices in place
            # (replaces 6 tiny N=64 matmuls + a vector reduce).
            HJ = S_OUT // 2
            for j0 in (0, HJ):
                nc.scalar.activation(
                    h[:, j0:j0 + HJ, :], pse[i][:, j0:j0 + HJ, :],
                    mybir.ActivationFunctionType.Relu,
                    bias=bias_sb[:],
                )
                nc.vector.tensor_tensor(
                    h[:, j0:j0 + HJ, :], h[:, j0:j0 + HJ, :],
                    wl_sb[:, i * S_OUT + j0:i * S_OUT + j0 + HJ, :]
                    .broadcast_to((NM, HJ, B_SUB)),
                    op=mybir.AluOpType.mult,
                )
            if use_pe and split_mm:
                HJ2 = S_OUT // 2
                nc.tensor.matmul(
                    lgp[:].broadcast_to((1, HJ2, B_SUB)), ones_sb[:],
                    h[:, 0:HJ2, :], start=first, stop=False,
                )
                nc.tensor.matmul(
                    lgp[:].broadcast_to((1, HJ2, B_SUB)), ones_sb[:],
                    h[:, HJ2:S_OUT, :], start=False, stop=last,
                )
            elif use_pe:
                nc.tensor.matmul(
                    lgp[:].broadcast_to((1, S_OUT, B_SUB)), ones_sb[:],
                    h[:], start=first, stop=last,
                )
            else:
                # keep the pending epilogue off the PE: partition-reduce on
                # gpsimd, then j-reduce + accumulate on vector
                pr = opool.tile([NM, S_OUT, B_SUB], F32, tag=f"pr{i % 2}",
                                name=f"pr{i}_{te}")
                nc.gpsimd.partition_all_reduce(
                    pr[:], h[:], channels=NM,
                    reduce_op=bass_isa.ReduceOp.add)
                if first:
                    nc.vector.tensor_reduce(
                        lgp[:], pr[0:1].transpose([0, 2, 1]),
                        axis=mybir.AxisListType.X, op=mybir.AluOpType.add)
                else:
                    prj = opool.tile([1, B_SUB], F32, tag=f"prj{i % 2}",
                                     name=f"prj{i}_{te}")
                    nc.vector.tensor_reduce(
                        prj[:], pr[0:1].transpose([0, 2, 1]),
                        axis=mybir.AxisListType.X, op=mybir.AluOpType.add)
                    nc.vector.tensor_tensor(
                        lgp[:], lgp[:], prj[:], op=mybir.AluOpType.add)

        def epilogue_fin(te, lgp):
            src_ap = lgp[:, 0, :] if len(lgp[:].shape) == 3 else lgp[:]
            ot = opool.tile([1, B_SUB], F32, tag="ot", name=f"ot_{te}")
            nc.scalar.activation(
                ot[:], src_ap,
                mybir.ActivationFunctionType.Sigmoid,
                bias=blin_sb[:],
            )
            nc.scalar.dma_start(out[:, te * B_SUB:(te + 1) * B_SUB],
                                ot[:], single_packet=True)

        def emit_epilogue():
            te, pse = pending.pop(0)
            lga = opool.tile([1, B_SUB], F32, tag="lga", name=f"lga_{te}")
            for i in range(S_OUT):
                epilogue_oi(te, pse, i, lga, use_pe=False)
            epilogue_fin(te, lga)

        def cell(t, a, oi, ps):
            xt = x_tiles[(t, a + oi)]
            twt = tw_tiles[a]
            for u in range(NU):
                nc.tensor.matmul(
                    ps[oi][:],
                    twt[:, u, :, 0:NM],
                    xt[:, u, :, :, :],
                    start=(a == 0 and u == 0),
                    stop=False,
                    perf_mode=DR,
                )
            if a % 2 == 0:
                nc.tensor.matmul(
                    ps[oi][:],
                    twt[0:NTAIL, NU, :, 0:NM],
                    xt[0:NTAIL, NU, :, :, :],
                    start=False,
                    stop=(a == KK - 1),
                    perf_mode=DR,
                )

        for t in range(N_SUB):
            ps = [
                pspool.tile([NM, S_OUT, B_SUB], F32, tag=f"ps{i}",
                            name=f"ps{i}_{t}")
                for i in range(S_OUT)
            ]
            if t == 0:
                # Ramp: ONE in-order HW queue (sync), issue strictly by
                # first-need time, fine-chunked so the PE starts on partial
                # tiles.  (Splitting across queues was tried and hurt: the
                # early DMA path is priority-limited, and parallel queues
                # steal bandwidth from the critical first tile.)
                get_tw(0, split=True)
                load_x(0, 0, chunks=[(0, 1), (1, 2), (2, 4), (4, 7),
                                     (7, 11), (11, NUX)])
                load_x(0, 1, chunks=[(0, 6), (6, 12), (12, NUX)])
                get_tw(1)
                load_x(0, 2, chunks=[(0, 6), (6, 12), (12, NUX)])
                get_tw(2)
                load_x(0, 3, chunks=[(0, 9), (9, NUX)])
                get_tw(3)
                load_x(0, 4, chunks=[(0, 9), (9, NUX)])
                get_tw(4)
                load_consts()
                load_x(0, RAMP_W, chunks=[(0, 9), (9, NUX)])
                load_x(0, S_OUT, chunks=[(0, 9), (9, NUX)])
                for ia in range(S_OUT + 1, NSLOT):
                    load_x(0, ia)
            else:
                get_tw(0)
                load_x(t, 0)
                for a in range(1, RAMP_W):
                    get_tw(a)
                    load_x(t, a)
                load_consts()
                load_x(t, RAMP_W)
                get_tw(RAMP_W)
                for ia in range(S_OUT, NSLOT):
                    load_x(t, ia)
            # wavefront ramp; filler dummies between diagonals keep the
            # PE busy across DMA waits so the clock gate never re-throttles
            RAMP_FILL = {0: 8, 1: 4, 2: 2} if t == 0 else {}
            for w in range(RAMP_W):
                for a in range(w + 1):
                    cell(t, a, w - a, ps)
                for _ in range(RAMP_FILL.get(w, 0)):
                    nc.tensor.matmul(dscr[:], dw[:, 0:NM], dx[:],
                                     start=True, stop=True)
                if w == 4 and pending:
                    emit_epilogue()
            # main loop
            for a in range(KK):
                get_tw(a)
                if a + 1 < KK:
                    get_tw(a + 1)
                nxt = a - 1 + NSLOT
                if a >= 1:
                    if nxt < S_IN:
                        load_x(t, nxt)
                    elif t + 1 < N_SUB:
                        load_x(t + 1, nxt - S_IN)
                if t == N_SUB - 1 and a == KK - 1:
                    # final iteration: ps[oi] completes right after cell
                    # (12, oi) — interleave epilogues with the next cells.
                    # oi=5 first so only ONE epilogue chain trails the
                    # last cell.
                    lgp = pspool.tile([1, 1, B_SUB], F32, tag="lg",
                                      name=f"lg_{t}")
                    order = [S_OUT - 1] + list(range(S_OUT - 1))
                    for k, oi in enumerate(order):
                        cell(t, a, oi, ps)
                        epilogue_oi(t, ps, oi, lgp, first=(k == 0),
                                    last=(k == S_OUT - 1),
                                    split_mm=(k == S_OUT - 1))
                    epilogue_fin(t, lgp)
                else:
                    for oi in range(S_OUT):
                        if a + oi >= RAMP_W:
                            cell(t, a, oi, ps)
            if t < N_SUB - 1:
                pending.append((t, ps))

        while pending:
            emit_epilogue()

    nc.compile()
    return nc


try:
    import ml_dtypes
    np_bf16 = ml_dtypes.bfloat16
    np_fp8 = ml_dtypes.float8_e4m3
except ImportError:  # pragma: no cover
    raise


def _prep_inputs(x, W4, b4, Wlin, blin):
    B = x.shape[0]
    r_main = np.arange(NU * 256).reshape(NU, 2, 128)
    boff_m = r_main // 324
    kl_m = r_main % 324
    kl_t = 208 + np.arange(NTAIL)

    xt = np.ascontiguousarray(
        x[:, 0].transpose(3, 4, 1, 2, 0)).reshape(324, S_IN, S_IN, B)
    xt8 = xt.astype(np_fp8)

    # main units: [u, g, p, j, ia, B] -> [ia, p, u, g, j, B]
    jj = boff_m[..., None] + np.arange(S_OUT)
    xm_all = xt8[kl_m[..., None], :, jj, :]
    xm_all = np.ascontiguousarray(xm_all.transpose(4, 2, 0, 1, 3, 5))

    # tail block: [ia, p, g, j, B]
    base = xt8[kl_t, :, 12:12 + S_OUT, :].transpose(1, 0, 2, 3)
    xtl_all = np.zeros((S_IN, NTAIL, 2, S_OUT, B), np_fp8)
    xtl_all[:, :, 0] = base
    xtl_all[:S_IN - 1, :, 1] = base[1:]

    # combined xf [ia, p, u(17), g, j, B]
    xf_all = np.zeros((S_IN, 128, NUX, 2, S_OUT, B), np_fp8)
    xf_all[:, :, :NU] = xm_all
    xf_all[:, :NTAIL, NU] = xtl_all

    T_flat = np.zeros((324, KK, KK, NM), np.float32)
    kl = np.arange(324)
    k_in_v = kl // S_IN
    l_in_v = kl % S_IN
    W4t = W4[:, 0].transpose(0, 3, 4, 1, 2)
    for ch in range(NCH):
        for kp in range(S_OUT):
            for lp in range(S_OUT):
                m = ch * 36 + kp * 6 + lp
                dk = k_in_v - kp
                dl = l_in_v - lp
                valid = (dk >= 0) & (dk < KK) & (dl >= 0) & (dl < KK)
                T_flat[valid, :, :, m] = W4t[ch, dk[valid], dl[valid]]
    Tq = (T_flat * WSCALE).astype(np_fp8)

    tfm_np = np.zeros((KK, 128, NUX, 2, NMP), np_fp8)
    tgt = Tq[kl_m, :, boff_m, :]             # [u, g, p, a, m]
    tfm_np[:, :, :NU, :, :NM] = tgt.transpose(3, 2, 0, 1, 4)
    tailT = Tq[kl_t, :, 12, :]               # [p, a, m]
    for a in range(0, KK, 2):
        tfm_np[a, :NTAIL, NU, 0, :NM] = tailT[:, a]
        if a + 1 < KK:
            tfm_np[a, :NTAIL, NU, 1, :NM] = tailT[:, a + 1]

    m_idx = np.arange(NM)
    ch_idx = m_idx // 36
    rem = m_idx % 36
    i_idx = np.arange(S_OUT)
    j_idx = np.arange(S_OUT)
    feat = (ch_idx[:, None, None] * 1296 + i_idx[None, :, None] * 216
            + j_idx[None, None, :] * 36 + rem[:, None, None])
    wl_np = (Wlin[0, feat].reshape(NM, S_OUT * S_OUT, 1)
             / WSCALE).astype(np_bf16)

    bias4_np = np.ascontiguousarray(
        (b4[m_idx // 36] * WSCALE).astype(np.float32).reshape(NM, 1))
    blin_np = np.asarray(blin, np.float32).reshape(1, 1)
    return xf_all, tfm_np, wl_np, bias4_np, blin_np


def kernel(x, W4, b4, Wlin, blin, _profile=False):
    x = np.asarray(x)
    W4 = np.asarray(W4)
    b4 = np.asarray(b4)
    Wlin = np.asarray(Wlin)
    blin = np.asarray(blin)

    xf_all, tfm_np, wl_np, bias4_np, blin_np = _prep_inputs(
        x, W4, b4, Wlin, blin)

    if "nc" not in _CACHE:
        _CACHE["nc"] = _build_nc()
    nc = _CACHE["nc"]

    in_maps = []
    for core in range(N_CORES):
        b0 = core * B_CORE
        xc = xf_all[..., b0:b0 + B_CORE].reshape(
            S_IN, 128, NUX, 2, S_OUT, N_SUB, B_SUB)
        xc = np.ascontiguousarray(xc.transpose(5, 0, 1, 2, 3, 4, 6))
        in_maps.append({
            "xf": xc,
            "tfm": tfm_np,
            "wl": wl_np,
            "bias4": bias4_np,
            "blin": blin_np,
        })

    res = run_bass_kernel_spmd(
        nc, in_maps, core_ids=list(range(N_CORES)), trace=_profile)
    outs = [res.results[i]["out"].reshape(B_CORE) for i in range(N_CORES)]
    full = np.concatenate(outs).reshape(B_TOTAL, 1).astype(np.float32)
    if _profile:
        return full, res
    return full



# revision 26
# speedup vs baseline: 1.0010x; 1.0004x over previous
"""Trainium2 Bass kernel for nn_ModelSimplest (4D conv -> relu -> linear -> sigmoid).

fp8 DoubleRow, folded-boff + wavefront ramp + a-paired tails + tuned DMA order.

Per (a, oi): 4212 contraction rows r = (boff, k, l), J-shift baked into SBUF
tiles.  16 full 256-row DR matmuls (u<16) + (even a) one 116x2-row DR tail
matmul pairing (a, a+1) -> 215 matmuls per (t, oi), 384 cols each.

x tile per (t, ia): [128, 17, 2, 6, 64] fp8 — u<16 main units
(rows r = u*256 + g*128 + p), u=16 = tail block (p<116: g=0 rows
(boff=12, kl=208+p) of ia, g=1 same rows of ia+1; zero above).
tfm[a]: [128, 17, 2, 112] — u16 (even a) = tail-pair stationary.

Schedule: wavefront ramp over cells a+oi<5, then a-major main loop.  DMA
issue order keeps all ramp stationaries ahead of later x tiles.

Perf notes (trace-driven, ~447us vs ~450us before):
- The PE is ~95% busy at the 162 ns/matmul stream floor (384 cols @2.4GHz
  + NX overhead); the algorithm family is at its optimum (partitions<=128
  force the (13/18)^2 kl zero-padding; Winograd/FFT lose on 18-point
  transform batching; column tiling only helps M<32; PSUM 16KB/partition
  blocks N=768 fusion), so the wins are schedule-level:
- Epilogue: relu (scalar, j-halves) -> x wl broadcast-mult (vector) -> ONE
  ones-matmul per (t, oi) whose stride-0 PSUM out-AP accumulates the 6
  j-slices in place (replaces 6 tiny N=64 matmuls each; ~11us less PE).
  The t=0 ("pending") epilogue runs its partition-reduce on gpsimd
  (partition_all_reduce, ~2.8us/call, latency-irrelevant there).
- HAM warm-up: N_DUMMY zero-input matmuls bridge the ~8us framework
  preamble + first-DMA latency so the PE clock gate opens (K=8/8) at
  ~11.5us and real cells start warm; RAMP_FILL dummies between wavefront
  diagonals absorb residual DMA waits so the gate never re-throttles
  (a >=3.4us idle window would halve the clock for ~7us).
- Ramp DMA: single in-order sync-engine HW queue, issue strictly by first
  need time, tiles u-chunked so matmuls start on partial tiles.  (Dual
  queue via the scalar engine was tried and is SLOWER: the early DMA path
  is serialization-limited, and parallel queues steal bandwidth from the
  critical first tile.)
- Tail: final a=12 processes oi=5 first so only one epilogue chain trails
  the last cell; final out-DMA issues from the scalar engine right after
  its sigmoid.  Remaining tail is ~6us: ~1.3us relu/mult chain + ~3.6us
  intrinsic DMA completion latency (priming does not help).
"""
import sys
from contextlib import ExitStack

import numpy as np

sys.path.insert(0, "/opt/trn_rl_repo")

from concourse import bacc, bass, bass_isa, mybir, tile  # noqa: E402
from concourse.bass_utils import run_bass_kernel_spmd  # noqa: E402

KK = 13
S_IN = 18
S_OUT = 6
N_CORES = 8
B_TOTAL = 1024
B_CORE = B_TOTAL // N_CORES
B_SUB = 64
N_SUB = B_CORE // B_SUB
NCH = 3
NM = NCH * S_OUT * S_OUT              # 108
NMP = 112
NROW = KK * S_IN * S_IN               # 4212
NU = 16
NUX = 17                              # 16 main units + tail slot
NTAIL = NROW - NU * 256               # 116
WSCALE = 256.0
NSLOT = 10
RAMP_W = 5
N_DUMMY = 18

F32 = mybir.dt.float32
BF16 = mybir.dt.bfloat16
FP8 = mybir.dt.float8e4
DR = mybir.MatmulPerfMode.DoubleRow

_CACHE = {}


def _build_nc():
    nc = bacc.Bacc(None, target_bir_lowering=False)

    xf = nc.dram_tensor("xf", [N_SUB, S_IN, 128, NUX, 2, S_OUT, B_SUB], FP8,
                        kind="ExternalInput")
    tfm = nc.dram_tensor("tfm", [KK, 128, NUX, 2, NMP], FP8,
                         kind="ExternalInput")
    wl = nc.dram_tensor("wl", [NM, S_OUT * S_OUT, 1], BF16, kind="ExternalInput")
    bias4 = nc.dram_tensor("bias4", [NM, 1], F32, kind="ExternalInput")
    blin = nc.dram_tensor("blin", [1, 1], F32, kind="ExternalInput")
    out = nc.dram_tensor("out", [1, B_CORE], F32, kind="ExternalOutput")

    with tile.TileContext(nc) as tc, ExitStack() as ctx:
        cpool = ctx.enter_context(tc.tile_pool(name="consts", bufs=1))
        wl_sb = cpool.tile([NM, S_OUT * S_OUT, 1], BF16)
        bias_sb = cpool.tile([NM, 1], F32)
        blin_sb = cpool.tile([1, 1], F32)
        ones_sb = cpool.tile([NM, 1], BF16)
        consts_loaded = []

        def load_consts():
            if not consts_loaded:
                nc.scalar.dma_start(wl_sb[:], wl[:])
                nc.scalar.dma_start(bias_sb[:], bias4[:])
                nc.scalar.dma_start(blin_sb[:], blin[:])
                consts_loaded.append(True)

        # HAM warm-up: zero-filled dummy DR matmuls keep the PE busy from
        # the end of the framework preamble (~7.6us) while the first x/tw
        # DMAs are in flight, so the clock gate opens (~3.4us of sustained
        # activity) before the real cells start instead of ~20us in.
        dpool = ctx.enter_context(tc.tile_pool(name="dummy", bufs=1))
        dw = dpool.tile([128, NMP], FP8)
        dx = dpool.tile([128, S_OUT * B_SUB], FP8)
        nc.vector.memset(dw[:], 0)
        nc.vector.memset(dx[:], 0)
        nc.vector.memset(ones_sb[:], 1.0)

        xpool = ctx.enter_context(tc.tile_pool(name="xs", bufs=1))
        twpool = ctx.enter_context(tc.tile_pool(name="tws", bufs=1))
        tw_tiles = {}

        def get_tw(a, split=False, eng=None):
            if a not in tw_tiles:
                eng = eng or nc.sync
                twt = twpool.tile([128, NUX, 2, NMP], FP8, tag=f"tfm{a}",
                                  name=f"tfm{a}")
                if split:
                    eng.dma_start(twt[:, 0:1], tfm[a, :, 0:1])
                    eng.dma_start(twt[:, 1:NUX], tfm[a, :, 1:NUX])
                else:
                    eng.dma_start(twt[:], tfm[a])
                tw_tiles[a] = twt
            return tw_tiles[a]

        x_tiles = {}

        def load_x(t, ia, chunks=None, eng=None):
            if (t, ia) in x_tiles:
                return
            eng = eng or nc.sync
            xt = xpool.tile([128, NUX, 2, S_OUT, B_SUB], FP8,
                            tag=f"x{ia % NSLOT}", name=f"x_{t}_{ia}")
            if chunks:
                for u0, u1 in chunks:
                    eng.dma_start(xt[:, u0:u1], xf[t, ia, :, u0:u1])
            else:
                eng.dma_start(xt[:], xf[t, ia])
            x_tiles[(t, ia)] = xt

        pspool = ctx.enter_context(
            tc.tile_pool(name="ps", bufs=1, space=bass.MemorySpace.PSUM))
        hpool = ctx.enter_context(tc.tile_pool(name="hs", bufs=1))
        opool = ctx.enter_context(tc.tile_pool(name="outs", bufs=2))

        # warm-up dummies: write-only scratch PSUM, zero inputs.
        dscr = pspool.tile([NM, S_OUT * B_SUB], F32, tag="dscr", name="dscr")
        for _ in range(N_DUMMY):
            nc.tensor.matmul(dscr[:], dw[:, 0:NM], dx[:],
                             start=True, stop=True)

        pending = []

        def epilogue_oi(te, pse, i, lgp, first=None, last=None,
                        use_pe=True, split_mm=False):
            if first is None:
                first = i == 0
            if last is None:
                last = i == S_OUT - 1

            h = hpool.tile([NM, S_OUT, B_SUB], BF16, tag=f"h{i}",
                           name=f"h{i}_{te}")
            # relu+scale in two j-halves so Act and DVE pipeline; then
            # collapse partitions AND j with a single ones-matmul whose
            # stride-0 PSUM out-AP accumulates the 6 j-slices in place
            # (replaces 6 tiny N=64 matmuls + a vector reduce).
            HJ = S_OUT // 2
            for j0 in (0, HJ):
                nc.scalar.activation(
                    h[:, j0:j0 + HJ, :], pse[i][:, j0:j0 + HJ, :],
                    mybir.ActivationFunctionType.Relu,
                    bias=bias_sb[:],
                )
                nc.vector.tensor_tensor(
                    h[:, j0:j0 + HJ, :], h[:, j0:j0 + HJ, :],
                    wl_sb[:, i * S_OUT + j0:i * S_OUT + j0 + HJ, :]
                    .broadcast_to((NM, HJ, B_SUB)),
                    op=mybir.AluOpType.mult,
                )
            if use_pe and split_mm:
                HJ2 = S_OUT // 2
                nc.tensor.matmul(
                    lgp[:].broadcast_to((1, HJ2, B_SUB)), ones_sb[:],
                    h[:, 0:HJ2, :], start=first, stop=False,
                )
                nc.tensor.matmul(
                    lgp[:].broadcast_to((1, HJ2, B_SUB)), ones_sb[:],
                    h[:, HJ2:S_OUT, :], start=False, stop=last,
                )
            elif use_pe:
                nc.tensor.matmul(
                    lgp[:].broadcast_to((1, S_OUT, B_SUB)), ones_sb[:],
                    h[:], start=first, stop=last,
                )
            else:
                # keep the pending epilogue off the PE: partition-reduce on
                # gpsimd, then j-reduce + accumulate on vector
                pr = opool.tile([NM, S_OUT, B_SUB], F32, tag=f"pr{i % 2}",
                                name=f"pr{i}_{te}")
                nc.gpsimd.partition_all_reduce(
                    pr[:], h[:], channels=NM,
                    reduce_op=bass_isa.ReduceOp.add)
                if first:
                    nc.vector.tensor_reduce(
                        lgp[:], pr[0:1].transpose([0, 2, 1]),
                        axis=mybir.AxisListType.X, op=mybir.AluOpType.add)
                else:
                    prj = opool.tile([1, B_SUB], F32, tag=f"prj{i % 2}",
                                     name=f"prj{i}_{te}")
                    nc.vector.tensor_reduce(
                        prj[:], pr[0:1].transpose([0, 2, 1]),
                        axis=mybir.AxisListType.X, op=mybir.AluOpType.add)
                    nc.vector.tensor_tensor(
                        lgp[:], lgp[:], prj[:], op=mybir.AluOpType.add)

        def epilogue_fin(te, lgp):
            src_ap = lgp[:, 0, :] if len(lgp[:].shape) == 3 else lgp[:]
            ot = opool.tile([1, B_SUB], F32, tag="ot", name=f"ot_{te}")
            nc.scalar.activation(
                ot[:], src_ap,
                mybir.ActivationFunctionType.Sigmoid,
                bias=blin_sb[:],
            )
            nc.scalar.dma_start(out[:, te * B_SUB:(te + 1) * B_SUB],
                                ot[:], single_packet=True)

        def emit_epilogue():
            te, pse = pending.pop(0)
            lga = opool.tile([1, B_SUB], F32, tag="lga", name=f"lga_{te}")
            for i in range(S_OUT):
                epilogue_oi(te, pse, i, lga, use_pe=False)
            epilogue_fin(te, lga)

        def cell(t, a, oi, ps):
            xt = x_tiles[(t, a + oi)]
            twt = tw_tiles[a]
            for u in range(NU):
                nc.tensor.matmul(
                    ps[oi][:],
                    twt[:, u, :, 0:NM],
                    xt[:, u, :, :, :],
                    start=(a == 0 and u == 0),
                    stop=False,
                    perf_mode=DR,
                )
            if a % 2 == 0:
                nc.tensor.matmul(
                    ps[oi][:],
                    twt[0:NTAIL, NU, :, 0:NM],
                    xt[0:NTAIL, NU, :, :, :],
                    start=False,
                    stop=(a == KK - 1),
                    perf_mode=DR,
                )

        for t in range(N_SUB):
            ps = [
                pspool.tile([NM, S_OUT, B_SUB], F32, tag=f"ps{i}",
                            name=f"ps{i}_{t}")
                for i in range(S_OUT)
            ]
            if t == 0:
                # Ramp: ONE in-order HW queue (sync), issue strictly by
                # first-need time, fine-chunked so the PE starts on partial
                # tiles.  (Splitting across queues was tried and hurt: the
                # early DMA path is priority-limited, and parallel queues
                # steal bandwidth from the critical first tile.)
                # tw0 and x00 interleaved by need: cell(0,0) reads u
                # ascending from BOTH tiles, so neither may block the
                # other's early chunks in the in-order queue.
                twt0 = twpool.tile([128, NUX, 2, NMP], FP8, tag="tfm0",
                                   name="tfm0")
                xt00 = xpool.tile([128, NUX, 2, S_OUT, B_SUB], FP8,
                                  tag="x0", name="x_0_0")
                nc.sync.dma_start(twt0[:, 0:1], tfm[0, :, 0:1])
                nc.sync.dma_start(xt00[:, 0:1], xf[0, 0, :, 0:1])
                nc.sync.dma_start(twt0[:, 1:5], tfm[0, :, 1:5])
                nc.sync.dma_start(xt00[:, 1:2], xf[0, 0, :, 1:2])
                nc.sync.dma_start(twt0[:, 5:11], tfm[0, :, 5:11])
                nc.sync.dma_start(xt00[:, 2:4], xf[0, 0, :, 2:4])
                nc.sync.dma_start(twt0[:, 11:NUX], tfm[0, :, 11:NUX])
                nc.sync.dma_start(xt00[:, 4:7], xf[0, 0, :, 4:7])
                nc.sync.dma_start(xt00[:, 7:11], xf[0, 0, :, 7:11])
                nc.sync.dma_start(xt00[:, 11:NUX], xf[0, 0, :, 11:NUX])
                tw_tiles[0] = twt0
                x_tiles[(0, 0)] = xt00
                load_x(0, 1, chunks=[(0, 6), (6, 12), (12, NUX)])
                get_tw(1)
                load_x(0, 2, chunks=[(0, 6), (6, 12), (12, NUX)])
                get_tw(2)
                load_x(0, 3, chunks=[(0, 9), (9, NUX)])
                get_tw(3)
                load_x(0, 4, chunks=[(0, 9), (9, NUX)])
                get_tw(4)
                load_consts()
                load_x(0, RAMP_W, chunks=[(0, 9), (9, NUX)])
                load_x(0, S_OUT, chunks=[(0, 9), (9, NUX)])
                for ia in range(S_OUT + 1, NSLOT):
                    load_x(0, ia)
            else:
                get_tw(0)
                load_x(t, 0)
                for a in range(1, RAMP_W):
                    get_tw(a)
                    load_x(t, a)
                load_consts()
                load_x(t, RAMP_W)
                get_tw(RAMP_W)
                for ia in range(S_OUT, NSLOT):
                    load_x(t, ia)
            # wavefront ramp; filler dummies between diagonals keep the
            # PE busy across DMA waits so the clock gate never re-throttles
            RAMP_FILL = {0: 8, 1: 4, 2: 2} if t == 0 else {}
            for w in range(RAMP_W):
                for a in range(w + 1):
                    cell(t, a, w - a, ps)
                for _ in range(RAMP_FILL.get(w, 0)):
                    nc.tensor.matmul(dscr[:], dw[:, 0:NM], dx[:],
                                     start=True, stop=True)
                if w == 4 and pending:
                    emit_epilogue()
            # main loop
            for a in range(KK):
                get_tw(a)
                if a + 1 < KK:
                    get_tw(a + 1)
                nxt = a - 1 + NSLOT
                if a >= 1:
                    if nxt < S_IN:
                        load_x(t, nxt)
                    elif t + 1 < N_SUB:
                        load_x(t + 1, nxt - S_IN)
                if t == N_SUB - 1 and a == KK - 1:
                    # final iteration: ps[oi] completes right after cell
                    # (12, oi) — interleave epilogues with the next cells.
                    # oi=5 first so only ONE epilogue chain trails the
                    # last cell.
                    lgp = pspool.tile([1, 1, B_SUB], F32, tag="lg",
                                      name=f"lg_{t}")
                    order = [S_OUT - 1] + list(range(S_OUT - 1))
                    for k, oi in enumerate(order):
                        cell(t, a, oi, ps)
                        epilogue_oi(t, ps, oi, lgp, first=(k == 0),
                                    last=(k == S_OUT - 1),
                                    split_mm=(k == S_OUT - 1))
                    epilogue_fin(t, lgp)
                else:
                    for oi in range(S_OUT):
                        if a + oi >= RAMP_W:
                            cell(t, a, oi, ps)
            if t < N_SUB - 1:
                pending.append((t, ps))

        while pending:
            emit_epilogue()

    nc.compile()
    return nc


try:
    import ml_dtypes
    np_bf16 = ml_dtypes.bfloat16
    np_fp8 = ml_dtypes.float8_e4m3
except ImportError:  # pragma: no cover
    raise


def _prep_inputs(x, W4, b4, Wlin, blin):
    B = x.shape[0]
    r_main = np.arange(NU * 256).reshape(NU, 2, 128)
    boff_m = r_main // 324
    kl_m = r_main % 324
    kl_t = 208 + np.arange(NTAIL)

    xt = np.ascontiguousarray(
        x[:, 0].transpose(3, 4, 1, 2, 0)).reshape(324, S_IN, S_IN, B)
    xt8 = xt.astype(np_fp8)

    # main units: [u, g, p, j, ia, B] -> [ia, p, u, g, j, B]
    jj = boff_m[..., None] + np.arange(S_OUT)
    xm_all = xt8[kl_m[..., None], :, jj, :]
    xm_all = np.ascontiguousarray(xm_all.transpose(4, 2, 0, 1, 3, 5))

    # tail block: [ia, p, g, j, B]
    base = xt8[kl_t, :, 12:12 + S_OUT, :].transpose(1, 0, 2, 3)
    xtl_all = np.zeros((S_IN, NTAIL, 2, S_OUT, B), np_fp8)
    xtl_all[:, :, 0] = base
    xtl_all[:S_IN - 1, :, 1] = base[1:]

    # combined xf [ia, p, u(17), g, j, B]
    xf_all = np.zeros((S_IN, 128, NUX, 2, S_OUT, B), np_fp8)
    xf_all[:, :, :NU] = xm_all
    xf_all[:, :NTAIL, NU] = xtl_all

    T_flat = np.zeros((324, KK, KK, NM), np.float32)
    kl = np.arange(324)
    k_in_v = kl // S_IN
    l_in_v = kl % S_IN
    W4t = W4[:, 0].transpose(0, 3, 4, 1, 2)
    for ch in range(NCH):
        for kp in range(S_OUT):
            for lp in range(S_OUT):
                m = ch * 36 + kp * 6 + lp
                dk = k_in_v - kp
                dl = l_in_v - lp
                valid = (dk >= 0) & (dk < KK) & (dl >= 0) & (dl < KK)
                T_flat[valid, :, :, m] = W4t[ch, dk[valid], dl[valid]]
    Tq = (T_flat * WSCALE).astype(np_fp8)

    tfm_np = np.zeros((KK, 128, NUX, 2, NMP), np_fp8)
    tgt = Tq[kl_m, :, boff_m, :]             # [u, g, p, a, m]
    tfm_np[:, :, :NU, :, :NM] = tgt.transpose(3, 2, 0, 1, 4)
    tailT = Tq[kl_t, :, 12, :]               # [p, a, m]
    for a in range(0, KK, 2):
        tfm_np[a, :NTAIL, NU, 0, :NM] = tailT[:, a]
        if a + 1 < KK:
            tfm_np[a, :NTAIL, NU, 1, :NM] = tailT[:, a + 1]

    m_idx = np.arange(NM)
    ch_idx = m_idx // 36
    rem = m_idx % 36
    i_idx = np.arange(S_OUT)
    j_idx = np.arange(S_OUT)
    feat = (ch_idx[:, None, None] * 1296 + i_idx[None, :, None] * 216
            + j_idx[None, None, :] * 36 + rem[:, None, None])
    wl_np = (Wlin[0, feat].reshape(NM, S_OUT * S_OUT, 1)
             / WSCALE).astype(np_bf16)

    bias4_np = np.ascontiguousarray(
        (b4[m_idx // 36] * WSCALE).astype(np.float32).reshape(NM, 1))
    blin_np = np.asarray(blin, np.float32).reshape(1, 1)
    return xf_all, tfm_np, wl_np, bias4_np, blin_np


def kernel(x, W4, b4, Wlin, blin, _profile=False):
    x = np.asarray(x)
    W4 = np.asarray(W4)
    b4 = np.asarray(b4)
    Wlin = np.asarray(Wlin)
    blin = np.asarray(blin)

    xf_all, tfm_np, wl_np, bias4_np, blin_np = _prep_inputs(
        x, W4, b4, Wlin, blin)

    if "nc" not in _CACHE:
        _CACHE["nc"] = _build_nc()
    nc = _CACHE["nc"]

    in_maps = []
    for core in range(N_CORES):
        b0 = core * B_CORE
        xc = xf_all[..., b0:b0 + B_CORE].reshape(
            S_IN, 128, NUX, 2, S_OUT, N_SUB, B_SUB)
        xc = np.ascontiguousarray(xc.transpose(5, 0, 1, 2, 3, 4, 6))
        in_maps.append({
            "xf": xc,
            "tfm": tfm_np,
            "wl": wl_np,
            "bias4": bias4_np,
            "blin": blin_np,
        })

    res = run_bass_kernel_spmd(
        nc, in_maps, core_ids=list(range(N_CORES)), trace=_profile)
    outs = [res.results[i]["out"].reshape(B_CORE) for i in range(N_CORES)]
    full = np.concatenate(outs).reshape(B_TOTAL, 1).astype(np.float32)
    if _profile:
        return full, res
    return full

